# revision 4
# baseline (speedup 1.0000x reference)
"""Multi-head attention (B=4, S=2048, d_model=1024, 16 heads) on 8 TRN2 NeuronCores.

Sharding: data-parallel over batch (4) x tensor-parallel over heads (2 groups
of 8) -> 8 cores. Each core computes, for its (batch b, head-group g):
  - Q/K/V projections in fp32r (full fp32 storage, 1 cyc/row matmuls)
  - scores^T [t, s] per head (fp16 operands), exp via ScalarE (scale=1/8)
  - softmax denominators via ones-matmul column sums, broadcast, normalize
  - attn^T (normalized, fp16) -> HBM; host transposes to [s, t]
  - attn @ V via col-packed fp16 matmuls; O-projection partial [S, d_model]
Host sums the two per-batch partials for the final `out` and casts/transposes
attn^T chunks into the full [B, H, S, S] fp32 attention output.
"""

import numpy as np

D_MODEL = 1024
N_HEADS = 16
D_K = D_MODEL // N_HEADS
SCALE = float(np.sqrt(D_K))
N_CORES = 8
HPC = N_HEADS // 2  # heads per core (8)
GDIM = HPC * D_K    # 512, head-group projection width

_cache = {}


def _build(S=2048, SB=None, bcast_gpsimd=True):
    """Build + compile the single-core SPMD bass program."""
    import concourse.bass as bass
    import concourse.tile as tile
    from concourse import bacc, mybir

    f32 = mybir.dt.float32
    f16 = mybir.dt.float16
    f32r = mybir.dt.float32r
    EXP = mybir.ActivationFunctionType.Exp
    ts = bass.ts

    if SB is None:
        SB = min(1024, S)
    assert S % 512 == 0 and SB % 512 == 0
    NJ = S // 128          # t-chunks per head
    NU = S // SB           # s-blocks per head
    NH = SB // 512         # 512-halves per s-block
    NC8 = D_MODEL // 128   # k-chunks in projections (8)

    nc = bacc.Bacc("TRN2", target_bir_lowering=False, debug=False,
                   num_devices=N_CORES)

    qT = nc.dram_tensor("qT", [D_MODEL, S], f32r, kind="ExternalInput").ap()
    kT = nc.dram_tensor("kT", [D_MODEL, S], f32r, kind="ExternalInput").ap()
    vT = nc.dram_tensor("vT", [D_MODEL, S], f32r, kind="ExternalInput").ap()
    wqT = nc.dram_tensor("wqT", [D_MODEL, GDIM], f32r, kind="ExternalInput").ap()
    wkT = nc.dram_tensor("wkT", [D_MODEL, GDIM], f32r, kind="ExternalInput").ap()
    wvT = nc.dram_tensor("wvT", [D_MODEL, GDIM], f32r, kind="ExternalInput").ap()
    woT = nc.dram_tensor("woT", [GDIM, D_MODEL], f16, kind="ExternalInput").ap()

    attnT_out = nc.dram_tensor("attnT", [HPC, S, S], f16, kind="ExternalOutput").ap()
    out_partial = nc.dram_tensor("out_partial", [S, D_MODEL], f32, kind="ExternalOutput").ap()

    with tile.TileContext(nc) as tc:
        with tc.tile_pool(name="pers", bufs=1) as pers:
            # [d-in-pair(128), pair(4), s/t] fp16 transposed projections
            qhT = pers.tile([128, 4, S], f16, tag="qhT")
            khT = pers.tile([128, 4, S], f16, tag="khT")
            vh = pers.tile([128, NJ, GDIM], f16, tag="vh")
            concatT = pers.tile([128, 4, S], f16, tag="concatT")
            ones16 = pers.tile([128, 1], f16, tag="ones16")
            nc.vector.memset(ones16[:], 1.0)

            # ---- projections (fp32r) ----
            with (
                tc.tile_pool(name="xT", bufs=1) as xp,
                tc.tile_pool(name="wp", bufs=2) as wp,
                tc.tile_pool(name="pproj", bufs=4, space="PSUM") as pp,
            ):
                for xin, win, kind in ((qT, wqT, "q"), (kT, wkT, "k"), (vT, wvT, "v")):
                    xt = xp.tile([128, NC8, S], f32r, tag="xT")
                    for c in range(NC8):
                        nc.sync.dma_start(xt[:, c, :], xin[c * 128:(c + 1) * 128, :])
                    wt = wp.tile([128, NC8, GDIM], f32r, tag="w")
                    nc.sync.dma_start(wt[:], win.rearrange("(c p) n -> p c n", p=128))
                    if kind in ("q", "k"):
                        dst = qhT if kind == "q" else khT
                        for D in range(4):
                            for sb in range(S // 512):
                                ps = pp.tile([128, 512], f32, tag="pp")
                                for c in range(NC8):
                                    nc.tensor.matmul(
                                        ps[:], wt[:, c, D * 128:(D + 1) * 128],
                                        xt[:, c, ts(sb, 512)],
                                        start=(c == 0), stop=(c == NC8 - 1))
                                nc.any.tensor_copy(dst[:, D, ts(sb, 512)], ps[:])
                    else:
                        for tb in range(NJ):
                            ps = pp.tile([128, 512], f32, tag="pp")
                            for c in range(NC8):
                                nc.tensor.matmul(
                                    ps[:], xt[:, c, ts(tb, 128)], wt[:, c, :],
                                    start=(c == 0), stop=(c == NC8 - 1))
                            nc.any.tensor_copy(vh[:, tb, :], ps[:])

            # ---- attention units: (pair p, s-block u) ----
            with (
                tc.tile_pool(name="expT", bufs=2 * NJ + 2) as ep,
                tc.tile_pool(name="pscA", bufs=1, space="PSUM") as pscA,
                tc.tile_pool(name="pscB", bufs=1, space="PSUM") as pscB,
                tc.tile_pool(name="pden", bufs=1, space="PSUM") as pd,
                tc.tile_pool(name="pav", bufs=2, space="PSUM") as pavp,
                tc.tile_pool(name="small", bufs=4) as sm,
            ):
                for p in range(4):
                    for u in range(NU):
                        den_ps = pd.tile([128, NH, 512], f32, tag="den")
                        etiles = {}
                        for j in range(NJ):
                            for a in range(2):
                                base = a * 64
                                pool = pscA if a == 0 else pscB
                                ps = pool.tile([128, SB], f32, tag=f"sc{a}")
                                for h2 in range(NH):
                                    nc.tensor.matmul(
                                        ps[:, ts(h2, 512)],
                                        khT[base:base + 64, p, ts(j, 128)],
                                        qhT[base:base + 64, p, u * SB + h2 * 512:
                                            u * SB + (h2 + 1) * 512],
                                        start=True, stop=True)
                                et = ep.tile([128, SB], f16, tag="expT")
                                nc.scalar.activation(et[:], ps[:], EXP, scale=1.0 / SCALE)
                                etiles[(j, a)] = et
                                for h2 in range(NH):
                                    nc.tensor.matmul(
                                        den_ps[a * 32:a * 32 + 1, h2, :],
                                        ones16[:], et[:, ts(h2, 512)],
                                        start=(j == 0), stop=(j == NJ - 1),
                                        skip_group_check=True)
                        # denominators -> reciprocal -> broadcast tiles
                        bcs = []
                        for a in range(2):
                            rec = sm.tile([1, SB], f32, tag="rec")
                            nc.vector.reciprocal(
                                rec[:], den_ps[a * 32:a * 32 + 1, :, :].rearrange("p a b -> p (a b)"))
                            rec16 = sm.tile([1, SB], f16, tag="rec16")
                            nc.vector.tensor_copy(rec16[:], rec[:])
                            bc = sm.tile([128, SB], f16, tag=f"bc{a}")
                            if bcast_gpsimd:
                                nc.gpsimd.partition_broadcast(bc[:], rec16[:])
                            else:
                                raise NotImplementedError
                            bcs.append(bc)
                        # normalize in place + write attn^T out
                        for j in range(NJ):
                            for a in range(2):
                                et = etiles[(j, a)]
                                nc.vector.tensor_mul(et[:], et[:], bcs[a][:])
                                nc.sync.dma_start(
                                    attnT_out[2 * p + a, ts(j, 128),
                                              u * SB:(u + 1) * SB], et[:])
                        # AV (col-packed pairs), accumulate over t-chunks
                        for b in range(NH):
                            avp = pavp.tile([128, 512], f32, tag="av")
                            for j in range(NJ):
                                for a in range(2):
                                    nc.tensor.matmul(
                                        avp[base_col(a)], vh[:, j, ts(2 * p + a, 64)],
                                        etiles[(j, a)][:, ts(b, 512)],
                                        start=(j == 0), stop=(j == NJ - 1),
                                        skip_group_check=True)
                            nc.any.tensor_copy(
                                concatT[:, p, u * SB + b * 512: u * SB + (b + 1) * 512],
                                avp[:])

            # ---- O-projection (fp16) ----
            with (
                tc.tile_pool(name="wo", bufs=1) as wop,
                tc.tile_pool(name="po", bufs=2, space="PSUM") as pop,
                tc.tile_pool(name="ost", bufs=3) as ostg,
            ):
                wo = wop.tile([128, 4, D_MODEL], f16, tag="wo")
                nc.sync.dma_start(wo[:], woT.rearrange("(c p) n -> p c n", p=128))
                for si in range(S // 128):
                    for eb in range(D_MODEL // 512):
                        ps = pop.tile([128, 512], f32, tag="po")
                        for c in range(4):
                            nc.tensor.matmul(
                                ps[:], concatT[:, c, ts(si, 128)],
                                wo[:, c, ts(eb, 512)],
                                start=(c == 0), stop=(c == 3))
                        ot = ostg.tile([128, 512], f32, tag="ot")
                        nc.any.tensor_copy(ot[:], ps[:])
                        nc.sync.dma_start(
                            out_partial[ts(si, 128), ts(eb, 512)], ot[:])

    nc.compile()
    return nc


def base_col(a):
    return (slice(0, 64), slice(None)) if a == 0 else (slice(64, 128), slice(None))


def _numpy_reference(q, k, v, w_q, b_q, w_k, b_k, w_v, b_v, w_o, b_o):
    B, S, _ = q.shape

    def proj(x, w, b):
        return (x @ w.T + b).reshape(B, S, N_HEADS, D_K).transpose(0, 2, 1, 3)

    qh = proj(q, w_q, b_q)
    kh = proj(k, w_k, b_k)
    vh = proj(v, w_v, b_v)
    scores = np.einsum("bhsd,bhtd->bhst", qh, kh) / SCALE
    scores -= scores.max(-1, keepdims=True)
    e = np.exp(scores)
    attn = e / e.sum(-1, keepdims=True)
    out = np.einsum("bhst,bhtd->bhsd", attn, vh)
    out = out.transpose(0, 2, 1, 3).reshape(B, S, D_MODEL)
    out = out @ w_o.T + b_o
    return out.astype(np.float32), attn.astype(np.float32)


def kernel(q, k, v, w_q, b_q, w_k, b_k, w_v, b_v, w_o, b_o):
    q = np.asarray(q, dtype=np.float32)
    k = np.asarray(k, dtype=np.float32)
    v = np.asarray(v, dtype=np.float32)
    w_q, w_k, w_v, w_o = (np.asarray(w, dtype=np.float32) for w in (w_q, w_k, w_v, w_o))
    b_q, b_k, b_v, b_o = (np.asarray(b, dtype=np.float32) for b in (b_q, b_k, b_v, b_o))

    # The device path folds no biases (they are zero in this problem's
    # setup_inputs). Exact-but-slow fallback if that ever changes.
    if any(np.abs(b).max() > 0 for b in (b_q, b_k, b_v)) :
        return _numpy_reference(q, k, v, w_q, b_q, w_k, b_k, w_v, b_v, w_o, b_o)

    from concourse.bass_utils import run_bass_kernel_spmd

    B, S, _ = q.shape
    if "nc" not in _cache:
        _cache["nc"] = _build(S=S)
    nc = _cache["nc"]

    qTs = [np.ascontiguousarray(q[b].T) for b in range(B)]
    kTs = [np.ascontiguousarray(k[b].T) for b in range(B)]
    vTs = [np.ascontiguousarray(v[b].T) for b in range(B)]
    wqTs = [np.ascontiguousarray(w_q[g * GDIM:(g + 1) * GDIM, :].T) for g in range(2)]
    wkTs = [np.ascontiguousarray(w_k[g * GDIM:(g + 1) * GDIM, :].T) for g in range(2)]
    wvTs = [np.ascontiguousarray(w_v[g * GDIM:(g + 1) * GDIM, :].T) for g in range(2)]
    woTs = [np.ascontiguousarray(w_o[:, g * GDIM:(g + 1) * GDIM].T).astype(np.float16)
            for g in range(2)]

    in_maps = []
    for core in range(N_CORES):
        b, g = core // 2, core % 2
        in_maps.append({
            "qT": qTs[b], "kT": kTs[b], "vT": vTs[b],
            "wqT": wqTs[g], "wkT": wkTs[g], "wvT": wvTs[g], "woT": woTs[g],
        })

    trace = bool(int(__import__("os").environ.get("BASS_KERNEL_TRACE", "0")))
    res = run_bass_kernel_spmd(nc, in_maps, core_ids=list(range(N_CORES)),
                               trace=trace)
    _cache["last_results"] = res

    out = np.empty((B, S, D_MODEL), dtype=np.float32)
    attn = np.empty((B, N_HEADS, S, S), dtype=np.float32)
    for b in range(B):
        out[b] = res.results[2 * b]["out_partial"] + res.results[2 * b + 1]["out_partial"]
        out[b] += b_o
    for core in range(N_CORES):
        b, g = core // 2, core % 2
        at = res.results[core]["attnT"]  # [HPC, t, s] f16
        attn[b, g * HPC:(g + 1) * HPC] = at.swapaxes(1, 2)
    return out, attn


# revision 6
# speedup vs baseline: 1.0430x; 1.0430x over previous
"""Multi-head attention (B=4, S=2048, d_model=1024, 16 heads) on 8 TRN2 NeuronCores.

Sharding: data-parallel over batch (4) x tensor-parallel over heads (2 groups
of 8) -> 8 cores. Each core computes, for its (batch b, head-group g):
  - Q/K/V projections in fp32r (full fp32 storage, 1 cyc/row matmuls)
  - scores^T [t, s] per head (fp16 operands), exp via ScalarE (scale=1/8)
  - softmax denominators via ones-matmul column sums, broadcast, normalize
  - attn^T (normalized, fp16) -> HBM; host transposes to [s, t]
  - attn @ V via col-packed fp16 matmuls; O-projection partial [S, d_model]
Host sums the two per-batch partials for the final `out` and casts/transposes
attn^T chunks into the full [B, H, S, S] fp32 attention output.
"""

import numpy as np

D_MODEL = 1024
N_HEADS = 16
D_K = D_MODEL // N_HEADS
SCALE = float(np.sqrt(D_K))
N_CORES = 8
HPC = N_HEADS // 2  # heads per core (8)
GDIM = HPC * D_K    # 512, head-group projection width

_cache = {}


def _build(S=2048, SB=None, bcast_gpsimd=True):
    """Build + compile the single-core SPMD bass program."""
    import concourse.bass as bass
    import concourse.tile as tile
    from concourse import bacc, mybir

    f32 = mybir.dt.float32
    f16 = mybir.dt.float16
    f32r = mybir.dt.float32r
    EXP = mybir.ActivationFunctionType.Exp
    ts = bass.ts

    if SB is None:
        SB = min(1024, S)
    assert S % 512 == 0 and SB % 512 == 0
    NJ = S // 128          # t-chunks per head
    NU = S // SB           # s-blocks per head
    NH = SB // 512         # 512-halves per s-block
    NC8 = D_MODEL // 128   # k-chunks in projections (8)

    nc = bacc.Bacc("TRN2", target_bir_lowering=False, debug=False,
                   num_devices=N_CORES)

    qT = nc.dram_tensor("qT", [D_MODEL, S], f32r, kind="ExternalInput").ap()
    kT = nc.dram_tensor("kT", [D_MODEL, S], f32r, kind="ExternalInput").ap()
    vT = nc.dram_tensor("vT", [D_MODEL, S], f32r, kind="ExternalInput").ap()
    wqT = nc.dram_tensor("wqT", [D_MODEL, GDIM], f32r, kind="ExternalInput").ap()
    wkT = nc.dram_tensor("wkT", [D_MODEL, GDIM], f32r, kind="ExternalInput").ap()
    wvT = nc.dram_tensor("wvT", [D_MODEL, GDIM], f32r, kind="ExternalInput").ap()
    woT = nc.dram_tensor("woT", [GDIM, D_MODEL], f16, kind="ExternalInput").ap()

    attnT_out = nc.dram_tensor("attnT", [HPC, S, S], f16, kind="ExternalOutput").ap()
    out_partial = nc.dram_tensor("out_partial", [S, D_MODEL], f32, kind="ExternalOutput").ap()

    with tile.TileContext(nc) as tc:
        with tc.tile_pool(name="pers", bufs=1) as pers:
            # [d-in-pair(128), pair(4), s/t] fp16 transposed projections
            qhT = pers.tile([128, 4, S], f16, tag="qhT")
            khT = pers.tile([128, 4, S], f16, tag="khT")
            vh = pers.tile([128, NJ, GDIM], f16, tag="vh")
            concatT = pers.tile([128, 4, S], f16, tag="concatT")
            ones16 = pers.tile([128, 1], f16, tag="ones16")
            nc.vector.memset(ones16[:], 1.0)

            # ---- projections (fp32r) ----
            with (
                tc.tile_pool(name="xT", bufs=1) as xp,
                tc.tile_pool(name="wp", bufs=2) as wp,
                tc.tile_pool(name="pproj", bufs=4, space="PSUM") as pp,
            ):
                for xin, win, kind in ((qT, wqT, "q"), (kT, wkT, "k"), (vT, wvT, "v")):
                    xt = xp.tile([128, NC8, S], f32r, tag="xT")
                    for c in range(NC8):
                        nc.sync.dma_start(xt[:, c, :], xin[c * 128:(c + 1) * 128, :])
                    wt = wp.tile([128, NC8, GDIM], f32r, tag="w")
                    nc.sync.dma_start(wt[:], win.rearrange("(c p) n -> p c n", p=128))
                    if kind in ("q", "k"):
                        dst = qhT if kind == "q" else khT
                        for D in range(4):
                            for sb in range(S // 512):
                                ps = pp.tile([128, 512], f32, tag="pp")
                                for c in range(NC8):
                                    nc.tensor.matmul(
                                        ps[:], wt[:, c, D * 128:(D + 1) * 128],
                                        xt[:, c, ts(sb, 512)],
                                        start=(c == 0), stop=(c == NC8 - 1))
                                nc.scalar.copy(dst[:, D, ts(sb, 512)], ps[:])
                    else:
                        for tb in range(NJ):
                            ps = pp.tile([128, 512], f32, tag="pp")
                            for c in range(NC8):
                                nc.tensor.matmul(
                                    ps[:], xt[:, c, ts(tb, 128)], wt[:, c, :],
                                    start=(c == 0), stop=(c == NC8 - 1))
                            nc.scalar.copy(vh[:, tb, :], ps[:])

            # ---- attention units: (pair p, s-block u) ----
            with (
                tc.tile_pool(name="expT", bufs=2 * NJ + 2) as ep,
                tc.tile_pool(name="pscA", bufs=1, space="PSUM") as pscA,
                tc.tile_pool(name="pscB", bufs=1, space="PSUM") as pscB,
                tc.tile_pool(name="pden", bufs=1, space="PSUM") as pd,
                tc.tile_pool(name="pav", bufs=2, space="PSUM") as pavp,
                tc.tile_pool(name="small", bufs=4) as sm,
            ):
                LN = mybir.ActivationFunctionType.Ln
                for p in range(4):
                    for u in range(NU):
                        den_ps = pd.tile([128, NH, 512], f32, tag="den")
                        etiles = {}
                        for j in range(NJ):
                            for a in range(2):
                                base = a * 64
                                pool = pscA if a == 0 else pscB
                                ps = pool.tile([128, SB], f32, tag=f"sc{a}")
                                for h2 in range(NH):
                                    nc.tensor.matmul(
                                        ps[:, ts(h2, 512)],
                                        khT[base:base + 64, p, ts(j, 128)],
                                        qhT[base:base + 64, p, u * SB + h2 * 512:
                                            u * SB + (h2 + 1) * 512],
                                        start=True, stop=True)
                                et = ep.tile([128, SB], f16, tag="expT")
                                nc.scalar.activation(et[:], ps[:], EXP, scale=1.0 / SCALE)
                                etiles[(j, a)] = et
                            # den column-sum matmuls: A/B adjacent so the
                            # M=1 tiles land in distinct col-groups and the
                            # PE runs each A/B pair concurrently.
                            for h2 in range(NH):
                                for a in range(2):
                                    nc.tensor.matmul(
                                        den_ps[a * 32:a * 32 + 1, h2, :],
                                        ones16[:], etiles[(j, a)][:, ts(h2, 512)],
                                        start=(j == 0), stop=(j == NJ - 1),
                                        skip_group_check=True)
                        # 1/den via exp(-ln(den)) on ScalarE (same table set
                        # as Exp), then broadcast across partitions (GpSimd).
                        bcs = []
                        for a in range(2):
                            lnd = sm.tile([1, SB], f32, tag="lnd")
                            nc.scalar.activation(
                                lnd[:],
                                den_ps[a * 32:a * 32 + 1, :, :].rearrange("p a b -> p (a b)"),
                                LN)
                            rec16 = sm.tile([1, SB], f16, tag="rec16")
                            nc.scalar.activation(rec16[:], lnd[:], EXP, scale=-1.0)
                            bc = sm.tile([128, SB], f16, tag=f"bc{a}")
                            nc.gpsimd.partition_broadcast(bc[:], rec16[:])
                            bcs.append(bc)
                        # normalize in place + write attn^T out
                        for j in range(NJ):
                            for a in range(2):
                                et = etiles[(j, a)]
                                nc.vector.tensor_mul(et[:], et[:], bcs[a][:])
                                nc.sync.dma_start(
                                    attnT_out[2 * p + a, ts(j, 128),
                                              u * SB:(u + 1) * SB], et[:])
                        # AV (col-packed pairs), accumulate over t-chunks
                        for b in range(NH):
                            avp = pavp.tile([128, 512], f32, tag="av")
                            for j in range(NJ):
                                for a in range(2):
                                    nc.tensor.matmul(
                                        avp[base_col(a)], vh[:, j, ts(2 * p + a, 64)],
                                        etiles[(j, a)][:, ts(b, 512)],
                                        start=(j == 0), stop=(j == NJ - 1),
                                        skip_group_check=True)
                            nc.vector.tensor_copy(
                                concatT[:, p, u * SB + b * 512: u * SB + (b + 1) * 512],
                                avp[:])

            # ---- O-projection (fp16) ----
            with (
                tc.tile_pool(name="wo", bufs=1) as wop,
                tc.tile_pool(name="po", bufs=2, space="PSUM") as pop,
                tc.tile_pool(name="ost", bufs=3) as ostg,
            ):
                wo = wop.tile([128, 4, D_MODEL], f16, tag="wo")
                nc.sync.dma_start(wo[:], woT.rearrange("(c p) n -> p c n", p=128))
                for si in range(S // 128):
                    for eb in range(D_MODEL // 512):
                        ps = pop.tile([128, 512], f32, tag="po")
                        for c in range(4):
                            nc.tensor.matmul(
                                ps[:], concatT[:, c, ts(si, 128)],
                                wo[:, c, ts(eb, 512)],
                                start=(c == 0), stop=(c == 3))
                        ot = ostg.tile([128, 512], f32, tag="ot")
                        nc.vector.tensor_copy(ot[:], ps[:])
                        nc.sync.dma_start(
                            out_partial[ts(si, 128), ts(eb, 512)], ot[:])

    nc.compile()
    return nc


def base_col(a):
    return (slice(0, 64), slice(None)) if a == 0 else (slice(64, 128), slice(None))


def _numpy_reference(q, k, v, w_q, b_q, w_k, b_k, w_v, b_v, w_o, b_o):
    B, S, _ = q.shape

    def proj(x, w, b):
        return (x @ w.T + b).reshape(B, S, N_HEADS, D_K).transpose(0, 2, 1, 3)

    qh = proj(q, w_q, b_q)
    kh = proj(k, w_k, b_k)
    vh = proj(v, w_v, b_v)
    scores = np.einsum("bhsd,bhtd->bhst", qh, kh) / SCALE
    scores -= scores.max(-1, keepdims=True)
    e = np.exp(scores)
    attn = e / e.sum(-1, keepdims=True)
    out = np.einsum("bhst,bhtd->bhsd", attn, vh)
    out = out.transpose(0, 2, 1, 3).reshape(B, S, D_MODEL)
    out = out @ w_o.T + b_o
    return out.astype(np.float32), attn.astype(np.float32)


def kernel(q, k, v, w_q, b_q, w_k, b_k, w_v, b_v, w_o, b_o):
    q = np.asarray(q, dtype=np.float32)
    k = np.asarray(k, dtype=np.float32)
    v = np.asarray(v, dtype=np.float32)
    w_q, w_k, w_v, w_o = (np.asarray(w, dtype=np.float32) for w in (w_q, w_k, w_v, w_o))
    b_q, b_k, b_v, b_o = (np.asarray(b, dtype=np.float32) for b in (b_q, b_k, b_v, b_o))

    # The device path folds no biases (they are zero in this problem's
    # setup_inputs). Exact-but-slow fallback if that ever changes.
    if any(np.abs(b).max() > 0 for b in (b_q, b_k, b_v)) :
        return _numpy_reference(q, k, v, w_q, b_q, w_k, b_k, w_v, b_v, w_o, b_o)

    from concourse.bass_utils import run_bass_kernel_spmd

    B, S, _ = q.shape
    if "nc" not in _cache:
        _cache["nc"] = _build(S=S)
    nc = _cache["nc"]

    qTs = [np.ascontiguousarray(q[b].T) for b in range(B)]
    kTs = [np.ascontiguousarray(k[b].T) for b in range(B)]
    vTs = [np.ascontiguousarray(v[b].T) for b in range(B)]
    wqTs = [np.ascontiguousarray(w_q[g * GDIM:(g + 1) * GDIM, :].T) for g in range(2)]
    wkTs = [np.ascontiguousarray(w_k[g * GDIM:(g + 1) * GDIM, :].T) for g in range(2)]
    wvTs = [np.ascontiguousarray(w_v[g * GDIM:(g + 1) * GDIM, :].T) for g in range(2)]
    woTs = [np.ascontiguousarray(w_o[:, g * GDIM:(g + 1) * GDIM].T).astype(np.float16)
            for g in range(2)]

    in_maps = []
    for core in range(N_CORES):
        b, g = core // 2, core % 2
        in_maps.append({
            "qT": qTs[b], "kT": kTs[b], "vT": vTs[b],
            "wqT": wqTs[g], "wkT": wkTs[g], "wvT": wvTs[g], "woT": woTs[g],
        })

    trace = bool(int(__import__("os").environ.get("BASS_KERNEL_TRACE", "0")))
    res = run_bass_kernel_spmd(nc, in_maps, core_ids=list(range(N_CORES)),
                               trace=trace)
    _cache["last_results"] = res

    out = np.empty((B, S, D_MODEL), dtype=np.float32)
    attn = np.empty((B, N_HEADS, S, S), dtype=np.float32)
    for b in range(B):
        out[b] = res.results[2 * b]["out_partial"] + res.results[2 * b + 1]["out_partial"]
        out[b] += b_o
    for core in range(N_CORES):
        b, g = core // 2, core % 2
        at = res.results[core]["attnT"]  # [HPC, t, s] f16
        attn[b, g * HPC:(g + 1) * HPC] = at.swapaxes(1, 2)
    return out, attn


# revision 12
# speedup vs baseline: 1.0592x; 1.0155x over previous
"""Multi-head attention (B=4, S=2048, d_model=1024, 16 heads) on 8 TRN2 NeuronCores.

Sharding: data-parallel over batch (4) x tensor-parallel over heads (2 groups
of 8) -> 8 cores. Each core computes, for its (batch b, head-group g):
  - Q/K/V projections in fp32r (full fp32 storage, 1 cyc/row matmuls)
  - scores^T [t, s] per head (fp16 operands), exp via ScalarE (scale=1/8)
  - softmax denominators via ones-matmul column sums, broadcast, normalize
  - attn^T (normalized, fp16) -> HBM; host transposes to [s, t]
  - attn @ V via col-packed fp16 matmuls; O-projection partial [S, d_model]
Host sums the two per-batch partials for the final `out` and casts/transposes
attn^T chunks into the full [B, H, S, S] fp32 attention output.
"""

import numpy as np

D_MODEL = 1024
N_HEADS = 16
D_K = D_MODEL // N_HEADS
SCALE = float(np.sqrt(D_K))
N_CORES = 8
HPC = N_HEADS // 2  # heads per core (8)
GDIM = HPC * D_K    # 512, head-group projection width

_cache = {}


def _build(S=2048, SB=None, bcast_gpsimd=True):
    """Build + compile the single-core SPMD bass program."""
    import concourse.bass as bass
    import concourse.tile as tile
    from concourse import bacc, mybir

    f32 = mybir.dt.float32
    f16 = mybir.dt.float16
    f32r = mybir.dt.float32r
    EXP = mybir.ActivationFunctionType.Exp
    ts = bass.ts

    if SB is None:
        SB = min(1024, S)
    assert S % 512 == 0 and SB % 512 == 0
    NJ = S // 128          # t-chunks per head
    NU = S // SB           # s-blocks per head
    NH = SB // 512         # 512-halves per s-block
    NC8 = D_MODEL // 128   # k-chunks in projections (8)

    nc = bacc.Bacc("TRN2", target_bir_lowering=False, debug=False,
                   num_devices=N_CORES)

    qT = nc.dram_tensor("qT", [D_MODEL, S], f32r, kind="ExternalInput").ap()
    kT = nc.dram_tensor("kT", [D_MODEL, S], f32r, kind="ExternalInput").ap()
    vT = nc.dram_tensor("vT", [D_MODEL, S], f32r, kind="ExternalInput").ap()
    wqT = nc.dram_tensor("wqT", [D_MODEL, GDIM], f32r, kind="ExternalInput").ap()
    wkT = nc.dram_tensor("wkT", [D_MODEL, GDIM], f32r, kind="ExternalInput").ap()
    wvT = nc.dram_tensor("wvT", [D_MODEL, GDIM], f32r, kind="ExternalInput").ap()
    woT = nc.dram_tensor("woT", [GDIM, D_MODEL], f16, kind="ExternalInput").ap()

    attnT_out = nc.dram_tensor("attnT", [HPC, S, S], f16, kind="ExternalOutput").ap()
    out_partial = nc.dram_tensor("out_partial", [S, D_MODEL], f32, kind="ExternalOutput").ap()

    with tile.TileContext(nc) as tc:
        with tc.tile_pool(name="pers", bufs=1) as pers:
            # [d-in-pair(128), pair(4), s/t] fp16 transposed projections
            qhT = pers.tile([128, 4, S], f16, tag="qhT")
            khT = pers.tile([128, 4, S], f16, tag="khT")
            vh = pers.tile([128, NJ, GDIM], f16, tag="vh")
            concatT = pers.tile([128, 4, S], f16, tag="concatT")
            ones16 = pers.tile([128, 1], f16, tag="ones16")
            nc.vector.memset(ones16[:], 1.0)

            # ---- projections (fp32r) ----
            # xT streamed in two 4-chunk halves (bufs=2) so the next x's DMA
            # overlaps the current x's matmuls.
            NCH = NC8 // 2
            with (
                tc.tile_pool(name="xT", bufs=2) as xp,
                tc.tile_pool(name="wp", bufs=2) as wp,
                tc.tile_pool(name="pproj", bufs=4, space="PSUM") as pp,
            ):
                for xin, win, kind in ((qT, wqT, "q"), (kT, wkT, "k"), (vT, wvT, "v")):
                    with nc.named_scope(f"proj_{kind}"):
                        halves = []
                        for hf in range(2):
                            xt = xp.tile([128, NCH, S], f32r, tag="xT")
                            for c in range(NCH):
                                nc.sync.dma_start(
                                    xt[:, c, :],
                                    xin[(hf * NCH + c) * 128:(hf * NCH + c + 1) * 128, :])
                            halves.append(xt)
                        wt = wp.tile([128, NC8, GDIM], f32r, tag="w")
                        nc.sync.dma_start(wt[:], win.rearrange("(c p) n -> p c n", p=128))
                        if kind in ("q", "k"):
                            dst = qhT if kind == "q" else khT
                            for D in range(4):
                                for sb in range(S // 512):
                                    ps = pp.tile([128, 512], f32, tag="pp")
                                    for c in range(NC8):
                                        nc.tensor.matmul(
                                            ps[:], wt[:, c, D * 128:(D + 1) * 128],
                                            halves[c // NCH][:, c % NCH, ts(sb, 512)],
                                            start=(c == 0), stop=(c == NC8 - 1))
                                    nc.scalar.copy(dst[:, D, ts(sb, 512)], ps[:])
                        else:
                            for tb in range(NJ):
                                ps = pp.tile([128, 512], f32, tag="pp")
                                for c in range(NC8):
                                    nc.tensor.matmul(
                                        ps[:], halves[c // NCH][:, c % NCH, ts(tb, 128)],
                                        wt[:, c, :],
                                        start=(c == 0), stop=(c == NC8 - 1))
                                nc.scalar.copy(vh[:, tb, :], ps[:])

            # ---- attention units: (s-block u outer, pair p inner) so the
            # O-projection for u's rows can overlap unit u+1 ----
            with (
                tc.tile_pool(name="expT", bufs=2 * NJ + 2) as ep,
                tc.tile_pool(name="pscA", bufs=1, space="PSUM") as pscA,
                tc.tile_pool(name="pscB", bufs=1, space="PSUM") as pscB,
                tc.tile_pool(name="pden", bufs=1, space="PSUM") as pd,
                tc.tile_pool(name="pav", bufs=2, space="PSUM") as pavp,
                tc.tile_pool(name="small", bufs=4) as sm,
                tc.tile_pool(name="wo", bufs=1) as wop,
                tc.tile_pool(name="ost", bufs=3) as ostg,
            ):
                wo = wop.tile([128, 4, D_MODEL], f16, tag="wo")
                nc.sync.dma_start(wo[:], woT.rearrange("(c p) n -> p c n", p=128))
                for u in range(NU):
                    for p in range(4):
                        sc_ctx = nc.named_scope(f"unit_{u}_{p}")
                        sc_ctx.__enter__()
                        den_ps = pd.tile([128, NH, 512], f32, tag="den")
                        etiles = {}
                        for j in range(NJ):
                            for a in range(2):
                                base = a * 64
                                pool = pscA if a == 0 else pscB
                                ps = pool.tile([128, SB], f32, tag=f"sc{a}")
                                for h2 in range(NH):
                                    nc.tensor.matmul(
                                        ps[:, ts(h2, 512)],
                                        khT[base:base + 64, p, ts(j, 128)],
                                        qhT[base:base + 64, p, u * SB + h2 * 512:
                                            u * SB + (h2 + 1) * 512],
                                        start=True, stop=True)
                                et = ep.tile([128, SB], f16, tag="expT")
                                nc.scalar.activation(et[:], ps[:], EXP, scale=1.0 / SCALE)
                                etiles[(j, a)] = et
                            # den column-sum matmuls: A/B adjacent so the
                            # M=1 tiles land in distinct col-groups and the
                            # PE runs each A/B pair concurrently.
                            for h2 in range(NH):
                                for a in range(2):
                                    nc.tensor.matmul(
                                        den_ps[a * 32:a * 32 + 1, h2, :],
                                        ones16[:], etiles[(j, a)][:, ts(h2, 512)],
                                        start=(j == 0), stop=(j == NJ - 1),
                                        skip_group_check=True)
                        # 1/den on DVE, broadcast across partitions (GpSimd).
                        bcs = []
                        for a in range(2):
                            rec = sm.tile([1, SB], f32, tag="rec")
                            nc.vector.reciprocal(
                                rec[:],
                                den_ps[a * 32:a * 32 + 1, :, :].rearrange("p a b -> p (a b)"))
                            rec16 = sm.tile([1, SB], f16, tag="rec16")
                            nc.vector.tensor_copy(rec16[:], rec[:])
                            bc = sm.tile([128, SB], f16, tag=f"bc{a}")
                            nc.gpsimd.partition_broadcast(bc[:], rec16[:])
                            bcs.append(bc)
                        # normalize in place + write attn^T out
                        for j in range(NJ):
                            for a in range(2):
                                et = etiles[(j, a)]
                                nc.vector.tensor_mul(et[:], et[:], bcs[a][:])
                                nc.sync.dma_start(
                                    attnT_out[2 * p + a, ts(j, 128),
                                              u * SB:(u + 1) * SB], et[:])
                        # AV (col-packed pairs), accumulate over t-chunks
                        for b in range(NH):
                            avp = pavp.tile([128, 512], f32, tag="av")
                            for j in range(NJ):
                                for a in range(2):
                                    nc.tensor.matmul(
                                        avp[base_col(a)], vh[:, j, ts(2 * p + a, 64)],
                                        etiles[(j, a)][:, ts(b, 512)],
                                        start=(j == 0), stop=(j == NJ - 1),
                                        skip_group_check=True)
                            nc.vector.tensor_copy(
                                concatT[:, p, u * SB + b * 512: u * SB + (b + 1) * 512],
                                avp[:])
                        sc_ctx.__exit__(None, None, None)
                    # ---- O-projection for this u's rows (fp16); shares the
                    # "av" PSUM tag so banks stay within 8 ----
                    with nc.named_scope(f"oproj_{u}"):
                        for si in range(u * (SB // 128), (u + 1) * (SB // 128)):
                            for eb in range(D_MODEL // 512):
                                ps = pavp.tile([128, 512], f32, tag="av")
                                for c in range(4):
                                    nc.tensor.matmul(
                                        ps[:], concatT[:, c, ts(si, 128)],
                                        wo[:, c, ts(eb, 512)],
                                        start=(c == 0), stop=(c == 3))
                                ot = ostg.tile([128, 512], f32, tag="ot")
                                nc.vector.tensor_copy(ot[:], ps[:])
                                nc.sync.dma_start(
                                    out_partial[ts(si, 128), ts(eb, 512)], ot[:])

    nc.compile()
    return nc


def base_col(a):
    return (slice(0, 64), slice(None)) if a == 0 else (slice(64, 128), slice(None))


def _numpy_reference(q, k, v, w_q, b_q, w_k, b_k, w_v, b_v, w_o, b_o):
    B, S, _ = q.shape

    def proj(x, w, b):
        return (x @ w.T + b).reshape(B, S, N_HEADS, D_K).transpose(0, 2, 1, 3)

    qh = proj(q, w_q, b_q)
    kh = proj(k, w_k, b_k)
    vh = proj(v, w_v, b_v)
    scores = np.einsum("bhsd,bhtd->bhst", qh, kh) / SCALE
    scores -= scores.max(-1, keepdims=True)
    e = np.exp(scores)
    attn = e / e.sum(-1, keepdims=True)
    out = np.einsum("bhst,bhtd->bhsd", attn, vh)
    out = out.transpose(0, 2, 1, 3).reshape(B, S, D_MODEL)
    out = out @ w_o.T + b_o
    return out.astype(np.float32), attn.astype(np.float32)


def kernel(q, k, v, w_q, b_q, w_k, b_k, w_v, b_v, w_o, b_o):
    q = np.asarray(q, dtype=np.float32)
    k = np.asarray(k, dtype=np.float32)
    v = np.asarray(v, dtype=np.float32)
    w_q, w_k, w_v, w_o = (np.asarray(w, dtype=np.float32) for w in (w_q, w_k, w_v, w_o))
    b_q, b_k, b_v, b_o = (np.asarray(b, dtype=np.float32) for b in (b_q, b_k, b_v, b_o))

    # The device path folds no biases (they are zero in this problem's
    # setup_inputs). Exact-but-slow fallback if that ever changes.
    if any(np.abs(b).max() > 0 for b in (b_q, b_k, b_v)) :
        return _numpy_reference(q, k, v, w_q, b_q, w_k, b_k, w_v, b_v, w_o, b_o)

    from concourse.bass_utils import run_bass_kernel_spmd

    B, S, _ = q.shape
    if "nc" not in _cache:
        _cache["nc"] = _build(S=S)
    nc = _cache["nc"]

    qTs = [np.ascontiguousarray(q[b].T) for b in range(B)]
    kTs = [np.ascontiguousarray(k[b].T) for b in range(B)]
    vTs = [np.ascontiguousarray(v[b].T) for b in range(B)]
    wqTs = [np.ascontiguousarray(w_q[g * GDIM:(g + 1) * GDIM, :].T) for g in range(2)]
    wkTs = [np.ascontiguousarray(w_k[g * GDIM:(g + 1) * GDIM, :].T) for g in range(2)]
    wvTs = [np.ascontiguousarray(w_v[g * GDIM:(g + 1) * GDIM, :].T) for g in range(2)]
    woTs = [np.ascontiguousarray(w_o[:, g * GDIM:(g + 1) * GDIM].T).astype(np.float16)
            for g in range(2)]

    in_maps = []
    for core in range(N_CORES):
        b, g = core // 2, core % 2
        in_maps.append({
            "qT": qTs[b], "kT": kTs[b], "vT": vTs[b],
            "wqT": wqTs[g], "wkT": wkTs[g], "wvT": wvTs[g], "woT": woTs[g],
        })

    trace = bool(int(__import__("os").environ.get("BASS_KERNEL_TRACE", "0")))
    res = run_bass_kernel_spmd(nc, in_maps, core_ids=list(range(N_CORES)),
                               trace=trace)
    _cache["last_results"] = res

    out = np.empty((B, S, D_MODEL), dtype=np.float32)
    attn = np.empty((B, N_HEADS, S, S), dtype=np.float32)
    for b in range(B):
        out[b] = res.results[2 * b]["out_partial"] + res.results[2 * b + 1]["out_partial"]
        out[b] += b_o
    for core in range(N_CORES):
        b, g = core // 2, core % 2
        at = res.results[core]["attnT"]  # [HPC, t, s] f16
        attn[b, g * HPC:(g + 1) * HPC] = at.swapaxes(1, 2)
    return out, attn
